# revision 33
# baseline (speedup 1.0000x reference)
"""Trainium2 Bass kernel: post-norm transformer block (8-head causal attention
d_model=64 + 64->2048->64 FFN), B=512 T=256, fp32 I/O.

Sharding: pure data-parallel over 8 NeuronCores - 64 sequences per core,
weights replicated. No collectives.

v2.2 (token-major, all-bf16 matmuls, instruction-count minimized):
  x loaded token-major fp32; bf16 copy -> 4 PE transposes -> xT feat-major,
  row-duplicated; Q/K each 2 row-packed concurrent matmuls; V token-major.
  scores: 4-head row-packed bf16 matmuls (K=8); exp on ScalarE; causal mask
  via gpsimd affine_select (Pool engine - keeps DVE free for PSUM work).
  o-matmuls use AUGMENTED v windows [v_h | 1...]:
  rows 0:8 of each 32-row group = o, rows 8:32 = row sums - the separate
  ones-matmul sums pass is gone.  Normalization: clamp -> reciprocal ->
  one small PE matmul against a 0/1 selector P broadcasts 1/sums from the
  sums rows to the o rows, then one multiply; all j-merged ([128, 2, 256]).
  proj token-major (pj[t,d] = o_blk^T @ wp) into ONE [128,4,64] accumulator;
  single residual add with fp32 x_tm; LN1 token-major (ONE bn_stats);
  LN1-apply emits bf16 hhat directly -> 4 transposes -> hh_fm row-dup.
  FFN1: 8x row-packed pairs, merged 2-chunk ReLU evicts (Scalar/DVE).
  FFN2 feat-major col-split: even chunks -> PE cols 0:64, odd -> 64:128,
  two concurrent N=512 accumulation chains; one bf16 evict; a 4-matmul
  "fold" against [I;I] turns z into token-major PSUM, and a 4-matmul
  g1diag accumulate adds the residual h = g1*hhat from hh_fm.  LN2
  token-major, direct DMA out.
"""
import numpy as np
import ml_dtypes

import concourse.bass as bass
import concourse.bacc as bacc
import concourse.tile as tile
from concourse import mybir
from concourse.bass_utils import run_bass_kernel_spmd

dt = mybir.dt
F32 = dt.float32
BF16 = dt.bfloat16
AF = mybir.ActivationFunctionType
OP = mybir.AluOpType

N_CORES = 8
B, T, D = 512, 256, 64
H, E = 8, 8
HID = 2048
S_PER_CORE = B // N_CORES  # 64 sequences/core
NPAIR = S_PER_CORE // 2    # 32 pair iterations
EPS = 1e-5

LAST_RESULTS = None  # test.py reads exec_time_ns from here
REPEAT = 1  # test-only: run the whole body N times in one NEFF for timing
_NC_CACHE = {}


def _build_bass():
    # All activation funcs used here (Exp, Ln, Relu, Copy) live in the one
    # table set "natural_log_exp_and_others"; restricting the table list pins
    # a single always-resident set (avoids ~2.7us ACT_TABLE_LOADs).
    import concourse.bacc as _bacc_mod
    _orig_gat = _bacc_mod.get_activation_tables

    def _one_set(arch):
        tabs = _orig_gat(arch)
        return {name: (fns if name == "natural_log_exp_and_others" else set())
                for name, fns in tabs.items()}

    _bacc_mod.get_activation_tables = _one_set
    try:
        return _build_bass_inner()
    finally:
        _bacc_mod.get_activation_tables = _orig_gat


def _build_bass_inner():
    nc = bacc.Bacc("TRN2", target_bir_lowering=False, debug=False)

    x_d = nc.dram_tensor("x", [S_PER_CORE * T, D], F32, kind="ExternalInput")
    wq_d = nc.dram_tensor("wq_sb", [128, 128], BF16, kind="ExternalInput")
    wk_d = nc.dram_tensor("wk_sb", [128, 128], BF16, kind="ExternalInput")
    wv_d = nc.dram_tensor("wv_sb", [D, D], BF16, kind="ExternalInput")
    wp_d = nc.dram_tensor("wp_sb", [2, 128, D], BF16, kind="ExternalInput")
    w1_d = nc.dram_tensor("w1_sb", [128, 8, 128], BF16, kind="ExternalInput")
    w2_d = nc.dram_tensor("w2_sb", [16, 128, D], BF16, kind="ExternalInput")
    id_d = nc.dram_tensor("ident_bf", [128, 128], BF16, kind="ExternalInput")
    pbc_d = nc.dram_tensor("pbc", [128, 128], BF16, kind="ExternalInput")
    fold_d = nc.dram_tensor("fold", [128, D], BF16, kind="ExternalInput")
    g1d_d = nc.dram_tensor("g1diag", [D, D], BF16, kind="ExternalInput")
    out_d = nc.dram_tensor("out", [S_PER_CORE * T, D], F32, kind="ExternalOutput")

    with tile.TileContext(nc) as tc:
        import contextlib
        with contextlib.ExitStack() as ctx:
            _build_body(ctx, tc, nc, x_d, wq_d, wk_d, wv_d, wp_d, w1_d, w2_d,
                        id_d, pbc_d, fold_d, g1d_d, out_d)
    nc.compile()
    return nc


def _build_body(ctx, tc, nc, x_d, wq_d, wk_d, wv_d, wp_d, w1_d, w2_d,
                id_d, pbc_d, fold_d, g1d_d, out_d):
    const = ctx.enter_context(tc.tile_pool(name="const", bufs=1))
    # PSUM: 8 banks.  ps = 2KB slots (4); psc = 4KB slots (2).  Row-packed
    # concurrent matmuls use the two banks of one psc tile; col-split/packed
    # matmuls share a bank (distinct partitions).
    ps = ctx.enter_context(tc.tile_pool(name="ps", bufs=4, space="PSUM"))
    psc = ctx.enter_context(tc.tile_pool(name="psc", bufs=2, space="PSUM"))
    sbA = ctx.enter_context(tc.tile_pool(name="sbA", bufs=4))
    sbB = ctx.enter_context(tc.tile_pool(name="sbB", bufs=8))
    sbH = ctx.enter_context(tc.tile_pool(name="sbH", bufs=2))

    # ---- constants / weights (persistent, distinct tags in bufs=1 pool) ----
    ident = const.tile([128, 128], BF16, tag="ident")
    nc.sync.dma_start(out=ident[:], in_=id_d.ap())
    pbc = const.tile([128, 128], BF16, tag="pbc")
    nc.sync.dma_start(out=pbc[:], in_=pbc_d.ap())
    fold = const.tile([128, D], BF16, tag="fold")
    nc.sync.dma_start(out=fold[:], in_=fold_d.ap())
    g1diag = const.tile([D, D], BF16, tag="g1diag")
    nc.sync.dma_start(out=g1diag[:], in_=g1d_d.ap())
    eps_t = const.tile([128, 1], F32, tag="eps_t")
    nc.vector.memset(eps_t[:], EPS)
    # augmented v: per head 32 cols = [v_h (8) | ones (24)]; the o-matmul
    # then emits row sums on rows 8:32 of each 32-row output group
    v_sb_bufs = [const.tile([128, 4, H, 32], BF16, tag=f"v_sb{i}",
                            name=f"v_sb{i}") for i in range(2)]
    for t in v_sb_bufs:
        nc.vector.memset(t[:, :, :, E:32], 1.0)

    wq_sb = const.tile([128, 128], BF16, tag="wq_sb")
    nc.sync.dma_start(out=wq_sb[:], in_=wq_d.ap())
    wk_sb = const.tile([128, 128], BF16, tag="wk_sb")
    nc.sync.dma_start(out=wk_sb[:], in_=wk_d.ap())
    wv_sb = const.tile([D, D], BF16, tag="wv_sb")
    nc.sync.dma_start(out=wv_sb[:], in_=wv_d.ap())
    wp_sb = const.tile([128, 2, D], BF16, tag="wp_sb")
    nc.sync.dma_start(out=wp_sb[:], in_=wp_d.ap().rearrange("r p m -> p r m"))
    w1_sb = const.tile([128, 8, 128], BF16, tag="w1_sb")
    nc.sync.dma_start(out=w1_sb[:], in_=w1_d.ap())
    w2_sb = const.tile([128, 16, D], BF16, tag="w2_sb")
    nc.sync.dma_start(out=w2_sb[:], in_=w2_d.ap().rearrange("c p m -> p c m"))

    x_ap = x_d.ap()
    out_ap = out_d.ap()

    # per-chunk 2D DMAs stay on the hardware DGE (a 3D strided DMA would
    # fall back to SWDGE).
    def load_pair(p):
        t = sbA.tile([128, 4, D], F32, tag="x_tm")
        for c in range(4):
            nc.sync.dma_start(out=t[:, c, :],
                              in_=x_ap[512 * p + 128 * c:512 * p + 128 * (c + 1)])
        return t

    def stage_a(x_tm):
        """x -> bf16 -> feat-major xT (row-dup) + Q/K (row-packed) + V."""
        st = {"x_tm": x_tm}
        x_bf = sbA.tile([128, 4, D], BF16, tag="x_bf")
        nc.gpsimd.tensor_copy(x_bf[:], x_tm[:])
        xT_ps = ps.tile([D, 4, 128], BF16, tag="ps")
        for c in range(4):
            nc.tensor.transpose(xT_ps[:, c, :], x_bf[:, c, :], ident[:])
        xT = sbA.tile([128, 4, 128], BF16, tag="xT")
        nc.vector.tensor_copy(xT[0:D], xT_ps[:])
        nc.sync.dma_start(out=xT[D:128], in_=xT[0:D])

        q_ps = psc.tile([128, 2, 512], F32, tag="sc")
        k_ps = psc.tile([128, 2, 512], F32, tag="sc")
        for r in range(2):  # both q matmuls first: r=0/r=1 run concurrently
            rows = slice(D * r, D * (r + 1))
            nc.tensor.matmul(q_ps[:, r, :], wq_sb[rows, :], xT[rows],
                             start=True, stop=True, tile_position=(D * r, 0))
        for r in range(2):
            rows = slice(D * r, D * (r + 1))
            nc.tensor.matmul(k_ps[:, r, :], wk_sb[rows, :], xT[rows],
                             start=True, stop=True, tile_position=(D * r, 0))
        q_sb = sbA.tile([128, 2, 512], BF16, tag="q_sb")
        nc.scalar.activation(q_sb[:], q_ps[:], AF.Copy)
        k_sb = sbA.tile([128, 2, 512], BF16, tag="k_sb")
        nc.scalar.activation(k_sb[:], k_ps[:], AF.Copy)
        st["q_sb"], st["k_sb"] = q_sb, k_sb

        v_ps = ps.tile([128, 4, E, E], F32, tag="ps")
        for c in range(4):
            nc.tensor.matmul(v_ps[:, c], xT[0:D, c, :], wv_sb[:],
                             start=True, stop=True, tile_position=(0, 0))
        v_sb = v_sb_bufs[stage_a.parity]
        stage_a.parity ^= 1
        nc.vector.tensor_copy(v_sb[:, :, :, 0:E], v_ps[:])
        st["v_sb"] = v_sb
        return st
    stage_a.parity = 0

    def stage_b1(st):
        """attention + token-major projection + residual-1."""
        q_sb, k_sb, v_sb, x_tm = st["q_sb"], st["k_sb"], st["v_sb"], st["x_tm"]
        pj = ps.tile([128, 4, D], F32, tag="ps")
        for r in range(2):  # head rounds (4 heads each)
            os_ps = ps.tile([128, 2, 256], F32, tag="ps")
            for j in range(2):  # seq in pair
                tcol = slice(256 * j, 256 * j + 256)
                t0 = slice(256 * j, 256 * j + 128)
                t1 = slice(256 * j + 128, 256 * j + 256)
                e_tiles = []
                for a in range(2):
                    sc = psc.tile([128, 2, 512], F32, tag="sc")
                    for b in range(2):
                        g = 2 * a + b
                        rg = slice(32 * g, 32 * g + 8)
                        nc.tensor.matmul(sc[:, b, 0:256], k_sb[rg, r, t0],
                                         q_sb[rg, r, tcol],
                                         start=True, stop=True,
                                         tile_position=(32 * g, 0))
                        nc.tensor.matmul(sc[:, b, 256:384], k_sb[rg, r, t1],
                                         q_sb[rg, r, t1],
                                         start=True, stop=True,
                                         tile_position=(32 * g, 0))
                    e = sbB.tile([128, 2, 384], BF16, tag="e")
                    nc.scalar.activation(e[:], sc[:, :, 0:384], AF.Exp)
                    # causal mask on gpsimd (keeps DVE free for PSUM work)
                    nc.gpsimd.affine_select(out=e[:, :, 0:128],
                                            in_=e[:, :, 0:128],
                                            compare_op=OP.is_ge, fill=0.0,
                                            base=0, pattern=[[0, 2], [1, 128]],
                                            channel_multiplier=-1)
                    nc.gpsimd.affine_select(out=e[:, :, 256:384],
                                            in_=e[:, :, 256:384],
                                            compare_op=OP.is_ge, fill=0.0,
                                            base=0, pattern=[[0, 2], [1, 128]],
                                            channel_multiplier=-1)
                    e_tiles.append(e)
                for g in range(4):
                    a, b = divmod(g, 2)
                    e0 = e_tiles[a][:, b, 0:256]
                    e1 = e_tiles[a][:, b, 256:384]
                    hh = 4 * r + g
                    cg = slice(32 * g, 32 * g + 32)
                    vA = v_sb[:, 2 * j, hh, :]
                    vB = v_sb[:, 2 * j + 1, hh, :]
                    nc.tensor.matmul(os_ps[cg, j, :], vA, e0,
                                     start=True, stop=False,
                                     tile_position=(0, 32 * g))
                    nc.tensor.matmul(os_ps[cg, j, 128:256], vB, e1,
                                     start=False, stop=True,
                                     tile_position=(0, 32 * g))
            # os rows 32g+0:8 = o_head, rows 32g+8:32 = sums replicas.
            # clamp avoids 1/0 on the o rows; pbc broadcasts 1/sums from
            # row 32g+8 to rows 32g..32g+8 (and 0 elsewhere).
            osc = sbB.tile([128, 2, 256], F32, tag="osc")
            nc.vector.tensor_scalar(out=osc[:], in0=os_ps[:], scalar1=1e-30,
                                    scalar2=None, op0=OP.max)
            rcp = sbB.tile([128, 2, 256], F32, tag="rcp")
            nc.vector.reciprocal_approx_fast(out=rcp[:], in_=osc[:])
            rcp_bf = sbB.tile([128, 2, 256], BF16, tag="rcp_bf")
            nc.gpsimd.tensor_copy(rcp_bf[:], rcp[:])
            rb_ps = ps.tile([128, 2, 256], F32, tag="ps")
            nc.tensor.matmul(rb_ps[:], pbc[:], rcp_bf[:],
                             start=True, stop=True, tile_position=(0, 0))
            rb = sbB.tile([128, 2, 256], F32, tag="rb")
            nc.scalar.activation(rb[:], rb_ps[:], AF.Copy)
            on = sbB.tile([128, 2, 256], BF16, tag="o_sb")
            nc.vector.tensor_tensor(out=on[:], in0=os_ps[:], in1=rb[:],
                                    op=OP.mult)
            st[f"o{r}"] = on
        # token-major projection: pj[t, d] accumulates o_r_blk^T @ wp_r
        for j in range(2):
            for tb in range(2):
                bcol = slice(128 * tb, 128 * tb + 128)
                for r in range(2):
                    nc.tensor.matmul(pj[:, 2 * j + tb, :],
                                     st[f"o{r}"][:, j, bcol], wp_sb[:, r, :],
                                     start=(r == 0), stop=(r == 1),
                                     tile_position=(0, 0))
        st.pop("o0"), st.pop("o1")
        # residual 1 (token-major, fp32 trunk)
        h_pre = sbA.tile([128, 4, D], F32, tag="h_pre")
        nc.vector.tensor_tensor(out=h_pre[:], in0=pj[:], in1=x_tm[:],
                                op=OP.add)
        st["h_pre"] = h_pre

    def stage_b2(st):
        """LN1 (token-major, no transposes) + bf16 hhat to feat-major."""
        h_pre = st.pop("h_pre")
        stt = sbB.tile([128, 4, 6], F32, tag="st")
        mv = sbB.tile([128, 4, 2], F32, tag="mv")
        for c in range(4):
            nc.vector.bn_stats(stt[:, c, :], h_pre[:, c, :])
            nc.vector.bn_aggr(mv[:, c, :], stt[:, c, :])
        # rstd = exp(-0.5*ln(var+eps)) on ScalarE (one ACT table set)
        sd = sbB.tile([128, 4], F32, tag="sd")
        nc.scalar.activation(sd[:], mv[:, :, 1], AF.Ln, bias=eps_t[:])
        rs = sbB.tile([128, 4], F32, tag="rs")
        nc.scalar.activation(rs[:], sd[:], AF.Exp, scale=-0.5)
        hh_bf = sbB.tile([128, 4, D], BF16, tag="hh_bf")
        for c in range(4):
            nc.gpsimd.tensor_scalar(out=hh_bf[:, c, :], in0=h_pre[:, c, :],
                                    scalar1=mv[:, c, 0:1],
                                    scalar2=rs[:, c:c + 1],
                                    op0=OP.subtract, op1=OP.mult)
        hhT_ps = ps.tile([D, 4, 128], BF16, tag="ps")
        for c in range(4):
            nc.tensor.transpose(hhT_ps[:, c, :], hh_bf[:, c, :], ident[:])
        hh_fm = sbA.tile([128, 4, 128], BF16, tag="hh_fm")
        nc.vector.tensor_copy(hh_fm[0:D], hhT_ps[:])
        nc.sync.dma_start(out=hh_fm[D:128], in_=hh_fm[0:D])
        st["hh_fm"] = hh_fm

    def stage_d1(st):
        """FFN1 row-packed + FFN2 feat-major col-split + token-major fold."""
        hh_fm = st["hh_fm"]
        hid = sbH.tile([128, 8, 2, 512], BF16, tag="hid")
        for ci in range(8):
            f = psc.tile([128, 2, 512], F32, tag="sc")
            nc.tensor.matmul(f[:, 0, :], w1_sb[0:D, ci, :], hh_fm[0:D],
                             start=True, stop=True, tile_position=(0, 0))
            nc.tensor.matmul(f[:, 1, :], w1_sb[D:128, ci, :], hh_fm[D:128],
                             start=True, stop=True, tile_position=(D, 0))
            # merged 2-chunk ReLU evict (PSUM source: Scalar/DVE only)
            if ci < 4:
                nc.scalar.activation(hid[:, ci, :, :], f[:], AF.Relu)
            else:
                nc.vector.tensor_scalar(out=hid[:, ci, :, :], in0=f[:],
                                        scalar1=0.0, scalar2=None, op0=OP.max)
        # FFN2: even chunks accumulate on PE cols 0:64 (zz rows 0:64), odd
        # on cols 64:128 - two concurrent large-N chains
        zz = ps.tile([128, 512], F32, tag="ps")
        for c in range(16):
            half = c & 1
            rows = slice(D * half, D * (half + 1))
            nc.tensor.matmul(zz[rows, :], w2_sb[:, c, :],
                             hid[:, c % 8, c // 8, :],
                             start=(c < 2), stop=(c >= 14),
                             tile_position=(0, D * half))
        zsb = sbA.tile([128, 512], BF16, tag="zsb")
        nc.vector.tensor_copy(zsb[:], zz[:])
        # fold halves to token-major z and accumulate residual h = g1*hhat
        z_tm = ps.tile([128, 4, D], F32, tag="ps")
        for c in range(4):
            bcol = slice(128 * c, 128 * c + 128)
            nc.tensor.matmul(z_tm[:, c, :], zsb[:, bcol], fold[:],
                             start=True, stop=False, tile_position=(0, 0))
            nc.tensor.matmul(z_tm[:, c, :], hh_fm[0:D, c, :], g1diag[:],
                             start=False, stop=True, tile_position=(0, 0))
        st["z_tm"] = z_tm

    def stage_d2(st, p):
        """LN2 (token-major) + store."""
        z_tm = st.pop("z_tm")
        st.pop("hh_fm")
        out_pre = sbA.tile([128, 4, D], F32, tag="out_pre")
        nc.vector.tensor_copy(out_pre[:], z_tm[:])
        st2 = sbB.tile([128, 4, 6], F32, tag="st2")
        mv2 = sbB.tile([128, 4, 2], F32, tag="mv2")
        for c in range(4):
            nc.vector.bn_stats(st2[:, c, :], out_pre[:, c, :])
            nc.vector.bn_aggr(mv2[:, c, :], st2[:, c, :])
        sd2 = sbB.tile([128, 4], F32, tag="sd2")
        nc.scalar.activation(sd2[:], mv2[:, :, 1], AF.Ln, bias=eps_t[:])
        rs2 = sbB.tile([128, 4], F32, tag="rs2")
        nc.scalar.activation(rs2[:], sd2[:], AF.Exp, scale=-0.5)
        out_sb = sbA.tile([128, 4, D], F32, tag="out_sb")
        for c in range(4):
            nc.gpsimd.tensor_scalar(out=out_sb[:, c, :], in0=out_pre[:, c, :],
                                    scalar1=mv2[:, c, 0:1],
                                    scalar2=rs2[:, c:c + 1],
                                    op0=OP.subtract, op1=OP.mult)
        for c in range(4):
            nc.sync.dma_start(
                out=out_ap[512 * p + 128 * c:512 * p + 128 * (c + 1)],
                in_=out_sb[:, c, :])

    # Software-pipelined emission (same skeleton as baseline): next pair's
    # load/QKV and attention are emitted between this pair's LN/FFN phases.
    pair_seq = [pp for _ in range(REPEAT) for pp in range(NPAIR)]
    n = len(pair_seq)
    sts = {0: stage_a(load_pair(pair_seq[0]))}
    stage_b1(sts[0])
    for i, p in enumerate(pair_seq):
        if i + 1 < n:
            sts[i + 1] = stage_a(load_pair(pair_seq[i + 1]))
        stage_b2(sts[i])
        stage_d1(sts[i])
        if i + 1 < n:
            stage_b1(sts[i + 1])
        stage_d2(sts.pop(i), p)


def _prep_weights(inputs):
    f32 = lambda a: np.ascontiguousarray(np.asarray(a, np.float32))
    bf = lambda a: np.ascontiguousarray(np.asarray(a).astype(ml_dtypes.bfloat16))
    Wq, Wk, Wv, Wp = (f32(inputs[k]) for k in ("Wq", "Wk", "Wv", "Wp"))
    g1, beta1, W1, b1 = (f32(inputs[k]) for k in ("g1", "beta1", "W1", "b1"))
    W2, b2 = f32(inputs["W2"]), f32(inputs["b2"])
    g2, beta2 = f32(inputs["g2"]), f32(inputs["beta2"])
    bq, bk, bv, bp = (f32(inputs[k]) for k in ("bq", "bk", "bv", "bp"))
    for name, b in (("bq", bq), ("bk", bk), ("bv", bv), ("bp", bp),
                    ("b1", b1), ("b2", b2), ("beta1", beta1), ("beta2", beta2)):
        assert not np.any(b), f"nonzero {name} not supported by this kernel build"
    assert np.all(g2 == 1.0), "non-unit g2 not supported by this kernel build"

    sc = 1.0 / np.sqrt(E)
    # spread head layout: rows/cols 32g:32g+8 hold head 4r+g
    wq_sb = np.zeros((128, 128), np.float32)
    wk_sb = np.zeros((128, 128), np.float32)
    wp_sb = np.zeros((2, 128, D), np.float32)
    for r in range(2):
        for g in range(4):
            h = 4 * r + g
            wq_sb[64 * r:64 * r + D, 32 * g:32 * g + 8] = Wq[h] * sc
            wk_sb[64 * r:64 * r + D, 32 * g:32 * g + 8] = Wk[h]
            wp_sb[r, 32 * g:32 * g + 8, :] = Wp[8 * h:8 * h + 8, :]
    wv_sb = Wv.transpose(1, 0, 2).reshape(D, D)  # [d, (h,e)]
    w1f = g1[:, None] * W1  # fold g1 [64, 2048]
    # partition-half split for row-packed FFN1: [128, 8, 128]
    w1_sb = np.zeros((128, 8, 128), np.float32)
    for ci in range(8):
        w1_sb[0:D, ci, :] = w1f[:, 128 * ci:128 * (ci + 1)]
        w1_sb[D:128, ci, :] = w1f[:, 128 * (ci + 8):128 * (ci + 9)]
    w2_sb = W2.reshape(16, 128, D)
    ident = np.eye(128, dtype=np.float32)
    pbc = np.zeros((128, 128), np.float32)
    for g in range(4):
        pbc[32 * g + 8, 32 * g:32 * g + 8] = 1.0
    fold = np.vstack([np.eye(D), np.eye(D)]).astype(np.float32)
    g1diag = np.diag(g1)
    return {
        "wq_sb": bf(wq_sb), "wk_sb": bf(wk_sb), "wv_sb": bf(wv_sb),
        "wp_sb": bf(wp_sb), "w1_sb": bf(w1_sb),
        "w2_sb": bf(np.ascontiguousarray(w2_sb)),
        "ident_bf": bf(ident),
        "pbc": bf(pbc), "fold": bf(fold), "g1diag": bf(g1diag),
    }


def kernel(**inputs) -> np.ndarray:
    global LAST_RESULTS
    x = np.ascontiguousarray(np.asarray(inputs["x"], np.float32))  # [512,256,64]
    weights = _prep_weights(inputs)

    nc = _NC_CACHE.get(REPEAT)
    if nc is None:
        nc = _NC_CACHE[REPEAT] = _build_bass()
    in_maps = []
    for core in range(N_CORES):
        shard = x[core * S_PER_CORE:(core + 1) * S_PER_CORE].reshape(
            S_PER_CORE * T, D)
        m = {"x": np.ascontiguousarray(shard)}
        m.update(weights)
        in_maps.append(m)

    res = run_bass_kernel_spmd(nc, in_maps, core_ids=list(range(N_CORES)))
    LAST_RESULTS = res
    out = np.concatenate(
        [res.results[c]["out"].reshape(S_PER_CORE, T, D) for c in range(N_CORES)],
        axis=0)
    return out
